# revision 32
# baseline (speedup 1.0000x reference)
"""Trainium2 Bass kernel for EnergyConditionedAtomAttention.

Strategy (8 NeuronCores, pure data-parallel, no collectives):
  - Batches are interleaved across cores: core c gets global batches
    {c, c+8, c+16, c+24}, reordered so batches with any attention edges
    come first.  All heavy compute (q/k/v MLPs, attention softmax,
    output MLP) runs on-device.
  - Host does featurization/layout only: h_abs gather, zemb lookup, RBF
    expansion, mask/distance scatter, weight re-layout, output gather.
  - Device layout is feature-major ([feature, token], features on
    partitions).  Scores are computed transposed (scores^T [n, e]) so
    the attention mask folds into the Exp activation as a per-partition
    bias; the softmax denominator falls out of the O-matmul via a
    ones-column appended to V — no vector-engine reductions anywhere.
  - Matmul dtypes: float32r for the MLPs (fp32 matmuls run 2-pass
    LOW_HIGH at 4 cycles/row; fp32r streams 1 row/cycle at free>=256),
    bfloat16 for K^T/Q^T/P^T/V (fast weight load + 1 row/cycle).
    PSUM accumulation is fp32 everywhere.
  - Fully-masked batch slots are exact by construction (exp(-1e9)=0 ->
    clamped denominator -> zeros) and are skipped when every core
    agrees the slot is masked; their q/k/v MLP columns are skipped too.
"""

import numpy as np

import concourse.bass as bass  # noqa: F401
import concourse.tile as tile
from concourse import bacc, mybir
from concourse.bass_utils import run_bass_kernel_spmd

# Problem constants (hardcoded per the task contract).
B, N, NE = 32, 256, 256
AD, ED, HID, LAT, RBF_N, ZE, NH = 128, 32, 256, 256, 16, 32, 8
HD = LAT // NH            # 32
CUT = 5.0
KIN = AD + ZE + 1 + RBF_N  # 177
NCORES = 8
BL = B // NCORES           # 4 local batches per core
T = BL * N                 # 1024 tokens per core
SCALE = float(HD) ** -0.5
F32 = mybir.dt.float32
FR = mybir.dt.float32r
BF = mybir.dt.bfloat16
NEG = -1.0e9

# Weight pair order (usage order) inside the packed [128, 11*512] tensor.
_W_PAIRS = ["kW1", "vW1", "qW1", "kW2", "vW2", "qW2",
            "kW3", "vW3", "qW3", "oW1", "oW2"]
_W_EARLY = 3  # first weight DMA covers this many pairs
_B_ORDER = ["kb1", "kb2", "kb3", "vb1", "vb2", "qb1", "qb2", "qb3", "ob1"]

# misc column map (single packed [128, 1062] input; row 0 carries the
# row-vector constants)
_MC_HABS = 0          # [128, 4]
_MC_BIASN = 4         # [128, 8]
_MC_ENV = 12          # [128, 8]
_MC_EFEAT = 20        # [128, 256]
_MC_IDENT = 276       # [128, 128]
_MC_BIAS = 404        # [128, 18] (f32 bits)
_MC_VB3 = 422         # row0 [1, 256]
_MC_OB2 = 678         # row0 [1, 256]
_MC_ONES = 934        # row0 [1, 128]
_MC_END = 1062

# When True, Silu is emitted as Sigmoid+mul (CoreSim doesn't implement Silu).
SIM_SAFE_SILU = False


def _pack_w_pair(w):
    """[K, 256] (K<=256) -> [128, 512] with K-chunks side by side, zero pad."""
    out = np.zeros((128, 512), np.float32)
    k = w.shape[0]
    out[: min(k, 128), 0:256] = w[0:128]
    if k > 128:
        out[: k - 128, 256:512] = w[128:]
    return out


def host_prep(inputs):
    """Build per-core input maps + program key. Pure numpy."""
    h = np.asarray(inputs["h"], np.float32)
    z = np.asarray(inputs["z"])
    mask = np.asarray(inputs["mask"])
    e_feat = np.asarray(inputs["e_feat"], np.float32)
    absorber_index = np.asarray(inputs["absorber_index"])
    att_dst = np.asarray(inputs["att_dst"])
    att_dist = np.asarray(inputs["att_dist"], np.float32)
    zemb = np.asarray(inputs["zemb"], np.float32)

    bar = np.arange(B)
    h_abs = h[bar, absorber_index]                      # [B, 128]

    amf = np.zeros((B * N,), bool)
    amf[att_dst] = True
    adf = np.zeros((B * N,), np.float32)
    adf[att_dst] = att_dist
    att_mask = amf.reshape(B, N) & mask
    d = adf.reshape(B, N)

    offsets = np.linspace(0.0, CUT, RBF_N, dtype=np.float32)
    coeff = np.float32(-0.5 / (offsets[1] - offsets[0]) ** 2)
    rbf = np.exp(coeff * (d[..., None] - offsets) ** 2).astype(np.float32)
    zr = zemb[z]                                        # [B, N, 32]
    is_abs = np.zeros((B, N), np.float32)
    is_abs[bar, absorber_index] = 1.0
    atom_static = np.concatenate(
        [h, zr, is_abs[..., None], rbf], axis=-1).astype(np.float32)  # [B,N,177]

    env = (0.5 * (np.cos(np.pi * d / CUT) + 1.0) * (d < CUT)).astype(np.float32)
    biasn = np.where(att_mask, 0.0, NEG).astype(np.float32)

    w_all = np.concatenate(
        [_pack_w_pair(np.asarray(inputs[nm], np.float32)) for nm in _W_PAIRS],
        axis=1)                                         # [128, 5632]
    # qW1's pair: cols 0-255 = rows 0:128 (h part), 256-511 = the 32-row
    # e_feat part zero-padded (exactly what _pack_w_pair does on [160, 256]).

    bias_all = np.zeros((128, 2 * len(_B_ORDER)), np.float32)
    for i, nm in enumerate(_B_ORDER):
        bias_all[:, 2 * i:2 * i + 2] = np.asarray(
            inputs[nm], np.float32).reshape(2, 128).T

    efeat_pad = np.zeros((128, 256), np.float32)
    efeat_pad[0:ED, :] = e_feat.T

    # Sort batch slots valid-first (identically on every core — the 8 cores
    # share one SPMD program).
    any_valid = att_mask.any(axis=1)
    pre_valid = [bool(any(any_valid[c + 8 * j] for c in range(NCORES)))
                 for j in range(BL)]
    order = sorted(range(BL), key=lambda j: not pre_valid[j])
    slot_valid = tuple(pre_valid[j] for j in order)

    in_maps = []
    cores_batches = []
    for c in range(NCORES):
        bs = [c + 8 * order[j] for j in range(BL)]
        cores_batches.append(bs)
        astT = atom_static[bs].reshape(T, KIN).T         # [177, 1024]
        kx = np.zeros((128, 2 * T), np.float32)
        kx[:, 0:T] = astT[0:128]
        kx[0:KIN - 128, T:2 * T] = astT[128:KIN]

        misc = np.zeros((128, _MC_END), np.float32)
        misc[:, _MC_HABS:_MC_HABS + 4] = h_abs[bs].T
        misc[:, _MC_BIASN:_MC_BIASN + 8] = biasn[bs].reshape(8, 128).T
        misc[:, _MC_ENV:_MC_ENV + 8] = env[bs].reshape(8, 128).T
        misc[:, _MC_EFEAT:_MC_EFEAT + 256] = efeat_pad
        misc[:, _MC_IDENT:_MC_IDENT + 128] = np.eye(128, dtype=np.float32)
        misc[:, _MC_BIAS:_MC_BIAS + 18] = bias_all
        misc[0, _MC_VB3:_MC_VB3 + 256] = np.asarray(inputs["vb3"], np.float32)
        misc[0, _MC_OB2:_MC_OB2 + 256] = np.asarray(inputs["ob2"], np.float32)
        misc[0, _MC_ONES:_MC_ONES + 128] = 1.0

        in_maps.append({"w_all": w_all, "kx": kx, "misc": misc})

    zero_b = frozenset(
        nm for nm in _B_ORDER + ["vb3", "ob2"]
        if not np.any(np.asarray(inputs[nm])))
    return in_maps, cores_batches, (slot_valid, zero_b)


def _silu(nc, sm_pool, out_ap, in_ap, bias):
    """out = silu(in + bias). bias is a [128,1] AP or float."""
    if not SIM_SAFE_SILU:
        nc.scalar.activation(out_ap, in_ap, mybir.ActivationFunctionType.Silu,
                             bias=bias)
    else:
        shp = [in_ap.shape[0], int(np.prod(in_ap.shape[1:]))]
        t1 = sm_pool.tile(shp, F32, tag="silu_pre", bufs=2, name="silu_pre")
        nc.scalar.activation(t1[:], in_ap, mybir.ActivationFunctionType.Identity,
                             bias=bias)
        t2 = sm_pool.tile(shp, F32, tag="silu_sig", bufs=2, name="silu_sig")
        nc.scalar.activation(t2[:], t1[:], mybir.ActivationFunctionType.Sigmoid)
        nc.vector.tensor_mul(out_ap, t1[:], t2[:])


def build_program(prog_key):
    slot_valid, zero_b = prog_key
    nc = bacc.Bacc("TRN2", target_bir_lowering=False, debug=False)
    w_all_d = nc.declare_dram_parameter("w_all", [128, 5632], FR, isOutput=False)
    kx_d = nc.declare_dram_parameter("kx", [128, 2048], FR, isOutput=False)
    misc_d = nc.declare_dram_parameter("misc", [128, _MC_END], FR, isOutput=False)
    out_d = nc.declare_dram_parameter("out", [T, LAT], F32, isOutput=True)

    widx = {nm: i for i, nm in enumerate(_W_PAIRS)}
    bidx = {nm: 2 * i for i, nm in enumerate(_B_ORDER)}
    nv = sum(bool(v) for v in slot_valid)   # valid slots are a prefix
    VT = nv * 256

    with tile.TileContext(nc) as tc:
        with (
            tc.tile_pool(name="const", bufs=1) as const,
            tc.tile_pool(name="acts", bufs=1) as acts,
            tc.tile_pool(name="ptp", bufs=max(2 * nv, 1)) as ptp,
            tc.tile_pool(name="small", bufs=2) as small,
            tc.tile_pool(name="ps", bufs=3, space="PSUM") as ps,
        ):
            # ---- loads: 4 input DMAs (kx, misc, weights x2) ----
            kx = const.tile([128, 2 * T], FR)
            nc.sync.dma_start(kx[:], kx_d[:])
            misc = const.tile([128, _MC_END], FR)
            nc.sync.dma_start(misc[:], misc_d[:])
            wall = const.tile([128, 5632], FR)
            esz = _W_EARLY * 512
            nc.sync.dma_start(wall[:, 0:esz], w_all_d[:, 0:esz])
            nc.sync.dma_start(wall[:, esz:5632], w_all_d[:, esz:5632])
            W = {nm: wall[:, widx[nm] * 512:(widx[nm] + 1) * 512]
                 for nm in _W_PAIRS}
            kx0 = kx[:, 0:T]
            kx1 = kx[:, T:2 * T]

            habs = misc[:, _MC_HABS:_MC_HABS + 4]
            biasn = misc[:, _MC_BIASN:_MC_BIASN + 8].bitcast(F32)
            env = misc[:, _MC_ENV:_MC_ENV + 8].bitcast(F32)
            efeat = misc[:, _MC_EFEAT:_MC_EFEAT + 256]
            ident = misc[:, _MC_IDENT:_MC_IDENT + 128]
            bias = misc[:, _MC_BIAS:_MC_BIAS + 18].bitcast(F32)
            vb3row = misc[0:1, _MC_VB3:_MC_VB3 + 256]
            ob2row = misc[0:1, _MC_OB2:_MC_OB2 + 256]
            ones1 = misc[0:1, _MC_ONES:_MC_ONES + 128]

            def bias_ap(bname, mc):
                if bname in zero_b:
                    return 0.0
                return bias[:, bidx[bname] + mc: bidx[bname] + mc + 1]

            def fm_layer(dst, rhs_tiles, wname, bname, act, ncols=T, c_lo=0):
                """Feature-major layer: dst[mc][:, c_lo:ncols] in 512-col chunks."""
                wt = W[wname]
                for mc in range(2):
                    for c0 in range(c_lo, ncols, 512):
                        cw = min(512, ncols - c0)
                        pm = ps.tile([128, 512], F32, tag="mlp", bufs=3, name="pm")
                        nkc = len(rhs_tiles)
                        for kc, rt in enumerate(rhs_tiles):
                            nc.tensor.matmul(
                                pm[:, 0:cw],
                                wt[:, kc * 256 + mc * 128: kc * 256 + mc * 128 + 128],
                                rt[:, c0:c0 + cw],
                                start=(kc == 0), stop=(kc == nkc - 1))
                        dap = dst[mc][:, c0:c0 + cw]
                        if act == "silu":
                            _silu(nc, small, dap, pm[:, 0:cw], bias_ap(bname, mc))
                        elif bname in zero_b:
                            # plain evacuation: keep it off the scalar engine
                            nc.vector.tensor_copy(dap, pm[:, 0:cw])
                        else:
                            nc.scalar.activation(
                                dap, pm[:, 0:cw],
                                mybir.ActivationFunctionType.Identity,
                                bias=bias_ap(bname, mc))

            def new_fm(name, dt=FR):
                return [acts.tile([128, T], dt, tag=f"{name}{i}", name=f"{name}{i}")
                        for i in range(2)]

            # ---- K/V/Q MLPs, layer-interleaved so the PE never drains
            # while an activation evacuates the previous layer ----
            ka1 = new_fm("ka1")
            va1 = new_fm("va1")
            qa1 = new_fm("qa1")
            fm_layer(ka1, [kx0, kx1], "kW1", "kb1", "silu", ncols=VT)
            fm_layer(va1, [kx0, kx1], "vW1", "vb1", "silu", ncols=VT)
            # q L1 exploits the shared input structure: every energy token of
            # batch j is [h_abs[j] | e_feat[e]], so L1 = (W1e^T e_feat^T)
            # shared across batches + per-batch per-partition bias
            # (W1h^T h_abs[j] + qb1) folded into the Silu activation.
            cb = const.tile([128, 8], F32)
            for mc in range(2):
                ep = ps.tile([128, 256], F32, tag="sm", bufs=4, name="ep")
                nc.tensor.matmul(ep[:],
                                 W["qW1"][:, 256 + mc * 128: 256 + mc * 128 + 128],
                                 efeat, start=True, stop=True)
                hp = ps.tile([128, 4], F32, tag="sm", bufs=4, name="hp")
                nc.tensor.matmul(hp[:], W["qW1"][:, mc * 128: mc * 128 + 128],
                                 habs, start=True, stop=True)
                nc.vector.tensor_scalar(cb[:, mc * 4:(mc + 1) * 4], hp[:],
                                        bias_ap("qb1", mc), None,
                                        mybir.AluOpType.add)
                for j in range(nv):
                    _silu(nc, small, qa1[mc][:, j * 256:(j + 1) * 256], ep[:],
                          cb[:, mc * 4 + j: mc * 4 + j + 1])

            ka2 = new_fm("ka2")
            va2 = new_fm("va2")
            qa2 = new_fm("qa2")
            fm_layer(ka2, ka1, "kW2", "kb2", "silu", ncols=VT)
            fm_layer(va2, va1, "vW2", "vb2", "silu", ncols=VT)
            fm_layer(qa2, qa1, "qW2", "qb2", "silu", ncols=VT)

            KT = new_fm("KT", BF)
            QT = new_fm("QT", BF)
            fm_layer(KT, ka2, "kW3", "kb3", "id", ncols=VT)
            # V L3 token-major with env scale and the softmax-denominator
            # ones-column (per-head blocks of 34 = 32 v | 1 ones | 1 pad).
            vaug = acts.tile([128, 8 * 272], BF)
            for t8 in range(2 * nv):
                pv = ps.tile([128, 256], F32, tag="sm", bufs=4, name="pv")
                if "vb3" not in zero_b:
                    nc.tensor.matmul(pv[:], ones1, vb3row, start=True, stop=False)
                for kc in range(2):
                    nc.tensor.matmul(
                        pv[:], va2[kc][:, t8 * 128:(t8 + 1) * 128],
                        W["vW3"][:, kc * 256:(kc + 1) * 256],
                        start=("vb3" in zero_b and kc == 0), stop=(kc == 1))
                seg3 = vaug[:, t8 * 272:(t8 + 1) * 272].rearrange(
                    "p (h w) -> p h w", h=8, w=34)
                src3 = pv[:].rearrange("p (h w) -> p h w", h=8, w=32)
                nc.vector.tensor_scalar(seg3[:, :, 0:32], src3,
                                        env[:, t8:t8 + 1], None,
                                        mybir.AluOpType.mult)
                nc.vector.tensor_scalar(seg3[:, :, 32:34], src3[:, :, 0:2],
                                        0.0, 1.0, mybir.AluOpType.mult,
                                        mybir.AluOpType.add)
            fm_layer(QT, qa2, "qW3", "qb3", "id", ncols=VT)

            # Matmul operands must start at partition 0/32/64 — restage the
            # 32-row head slices that sit at offset 96 (heads 3 and 7).
            kh = {}
            qh = {}
            for hh in (3, 7) if nv else ():
                kt = acts.tile([32, T], BF, tag=f"kh{hh}", name=f"kh{hh}")
                nc.vector.tensor_copy(kt[:, 0:VT], KT[hh // 4][96:128, 0:VT])
                kh[hh] = kt
                qt = acts.tile([32, T], BF, tag=f"qh{hh}", name=f"qh{hh}")
                nc.vector.tensor_copy(qt[:, 0:VT], QT[hh // 4][96:128, 0:VT])
                qh[hh] = qt

            def kt_ap(hh, c0, c1):
                if hh % 4 == 3:
                    return kh[hh][:, c0:c1]
                r0 = (hh % 4) * 32
                return KT[hh // 4][r0:r0 + 32, c0:c1]

            def qt_ap(hh, c0, c1):
                if hh % 4 == 3:
                    return qh[hh][:, c0:c1]
                r0 = (hh % 4) * 32
                return QT[hh // 4][r0:r0 + 32, c0:c1]

            # ---- attention ----
            # Phase 1: all score matmuls + masked exp (PT, bf16); restaged
            # heads (3, 7) last so their staging copies overlap.  Phase 2:
            # O-matmuls / normalize / transpose, software-pipelined one group
            # deep so the PE streams the next O-group while the DVE
            # normalizes this one.
            aoT = new_fm("aoT")   # feature-major attention output [lat, tok]
            for j in range(nv, BL):
                for lh in range(2):
                    nc.vector.tensor_scalar(
                        aoT[lh][:, j * 256:(j + 1) * 256],
                        kx0[:, 0:256], 0.0, None, mybir.AluOpType.mult)
            PT = {}
            for j in range(nv):
                for n2 in range(2):
                    ptw = ptp.tile([128, 8 * 256], BF, tag="pt",
                                   bufs=max(2 * nv, 1), name="ptw")
                    PT[j, n2] = ptw
                    for hh in (0, 1, 2, 4, 5, 6, 3, 7):
                        sp = ps.tile([128, 256], F32, tag="sm", bufs=4, name="sp")
                        nc.tensor.matmul(
                            sp[:],
                            kt_ap(hh, j * 256 + n2 * 128, j * 256 + n2 * 128 + 128),
                            qt_ap(hh, j * 256, (j + 1) * 256),
                            start=True, stop=True)
                        nc.scalar.activation(
                            ptw[:, hh * 256:(hh + 1) * 256], sp[:],
                            mybir.ActivationFunctionType.Exp,
                            bias=biasn[:, j * 2 + n2: j * 2 + n2 + 1], scale=SCALE)

            def o_group(j, ec):
                po = ps.tile([128, 272], F32, tag="sm", bufs=4, name="po")
                for hh in range(NH):
                    for n2 in range(2):
                        nc.tensor.matmul(
                            po[:, hh * 34:(hh + 1) * 34],
                            PT[j, n2][:, hh * 256 + ec * 128: hh * 256 + ec * 128 + 128],
                            vaug[:, (j * 2 + n2) * 272 + hh * 34:
                                   (j * 2 + n2) * 272 + (hh + 1) * 34],
                            start=(n2 == 0), stop=(n2 == 1))
                return po

            def o_finish(j, ec, po):
                dcol = po[:].rearrange("p (h w) -> p h w", h=8, w=34)[:, :, 32]
                mx = small.tile([128, 8], F32, tag="mx", name="mx")
                nc.vector.tensor_scalar_max(mx[:], dcol, 1e-8)
                rd = small.tile([128, 8], F32, tag="rd", name="rd")
                nc.vector.reciprocal(rd[:], mx[:])
                at = small.tile([128, 256], FR, tag="at", name="at")
                for hh in range(NH):
                    nc.vector.tensor_scalar(
                        at[:, hh * 32:(hh + 1) * 32],
                        po[:, hh * 34: hh * 34 + 32],
                        rd[:, hh:hh + 1], None, mybir.AluOpType.mult)
                for lh in range(2):
                    tp = ps.tile([128, 128], FR, tag="sm", bufs=4, name="tp")
                    nc.tensor.transpose(tp[:], at[:, lh * 128:(lh + 1) * 128],
                                        ident)
                    nc.vector.tensor_copy(
                        aoT[lh][:, j * 256 + ec * 128: j * 256 + ec * 128 + 128],
                        tp[:])

            prev = None
            for j in range(nv):
                for ec in range(2):
                    po = o_group(j, ec)
                    if prev is not None:
                        o_finish(*prev)
                    prev = (j, ec, po)
            if prev is not None:
                o_finish(*prev)

            # ---- O MLP (full width: masked slots produce the bias constant) ----
            oa1 = new_fm("oa1")
            fm_layer(oa1, aoT, "oW1", "ob1", "silu")
            for t8 in range(8):
                pz = ps.tile([128, 256], F32, tag="sm", bufs=4, name="pz")
                if "ob2" not in zero_b:
                    nc.tensor.matmul(pz[:], ones1, ob2row, start=True, stop=False)
                for kc in range(2):
                    nc.tensor.matmul(
                        pz[:], oa1[kc][:, t8 * 128:(t8 + 1) * 128],
                        W["oW2"][:, kc * 256:(kc + 1) * 256],
                        start=("ob2" in zero_b and kc == 0), stop=(kc == 1))
                ob = small.tile([128, 256], F32, tag="ob", name="ob")
                nc.vector.tensor_copy(ob[:], pz[:])
                nc.sync.dma_start(out_d[t8 * 128:(t8 + 1) * 128, :], ob[:])

    nc.compile()
    return nc


def run(inputs, trace=False):
    in_maps, cores_batches, prog_key = host_prep(inputs)
    nc = build_program(prog_key)
    res = run_bass_kernel_spmd(nc, in_maps, core_ids=list(range(NCORES)),
                               trace=trace)
    out = np.zeros((B, NE, LAT), np.float32)
    for c, bs in enumerate(cores_batches):
        oc = res.results[c]["out"].reshape(BL, NE, LAT)
        for j, gb in enumerate(bs):
            out[gb] = oc[j]
    return out, res


def kernel(**inputs) -> np.ndarray:
    out, _ = run(inputs)
    return out
